# revision 1
# baseline (speedup 1.0000x reference)
"""Trainium2 Bass kernel for 3x3 SAME conv: B=8, Cin=Cout=16, 1024x1024, fp32.

Reference semantics:
  x (8,16,1024,1024) fp32 raw-reshaped to NHWC (8,1024,1024,16);
  y = conv2d_3x3_SAME(x_nhwc, W[3,3,16,16] HWIO) -> NCHW (8,16,1024,1024).

Per-core (batch-parallel, image b -> core b):
  - W axis blocked into out-blocks of S=6; in-blocks of QN=8 pixels at offset -1.
  - K-tiles [(q,ci)=128 partitions, h free] built by PE transposes of naturally
    loaded [h 128, (q 8, ci 16)] tiles (512B-run DMAs, 1.33x read amplification).
  - Conv = 3 accumulating float32r matmuls per (jw, h-chunk<=512): lhsT =
    Toeplitz T_ky [128, 96=(co,op)], rhs = K-tile shifted by ky along h.
  - Output: psum [96, h] -> SBUF staging -> PE transposes [96,128] -> packed
    [h 128, (co, hc, jw, op)] staging -> NCHW DMA (contiguous w-runs).
"""
import numpy as np

C = 16          # channels
S = 6           # out-block width
QN = 8          # in-block width
KS = 3
M = 96          # = C * S


def _build_conv_program(H, W, n_cores, G=16, loop_count=1):
    import concourse.bacc as bacc
    import concourse.tile as tile
    import concourse.mybir as mybir

    dt = mybir.dt
    JW = (W + S - 1) // S
    HC = H // 128                   # 128-row chunks
    assert H % 512 == 0
    MMC = H // 512                  # matmul chunks of 512

    nc = bacc.Bacc("TRN2", target_bir_lowering=False, debug=False,
                   num_devices=n_cores)
    x_d = nc.dram_tensor("x", [H, W * C], dt.float32, kind="ExternalInput")
    t_d = nc.dram_tensor("tmat", [128, KS * M], dt.float32r, kind="ExternalInput")
    i_d = nc.dram_tensor("ident", [128, 128], dt.float32, kind="ExternalInput")
    y_d = nc.dram_tensor("y", [C, H, W], dt.float32, kind="ExternalOutput")
    # x as [p, hc, w, ci]
    x_v = x_d.ap().rearrange("(hc p) (w ci) -> p hc w ci", p=128, ci=C)
    # y as [p(h-in-chunk), hc, co, w]
    y_v = y_d.ap().rearrange("co (hc p) w -> p hc co w", p=128)

    groups = [(g0, min(G, JW - g0)) for g0 in range(0, JW, G)]

    with tile.TileContext(nc) as tc:
        with tc.tile_pool(name="const", bufs=1) as cpool, \
             tc.tile_pool(name="inp", bufs=3) as inp_pool, \
             tc.tile_pool(name="ktp", bufs=2) as kt_pool, \
             tc.tile_pool(name="stp", bufs=3) as st_pool, \
             tc.tile_pool(name="ostp", bufs=2) as ost_pool, \
             tc.tile_pool(name="tr_ps", bufs=2, space="PSUM") as tr_ps, \
             tc.tile_pool(name="mm_ps", bufs=2, space="PSUM") as mm_ps, \
             tc.tile_pool(name="ot_ps", bufs=2, space="PSUM") as ot_ps:

            tmat = cpool.tile([128, KS * M], dt.float32r)
            ident = cpool.tile([128, 128], dt.float32)
            nc.sync.dma_start(tmat[:], t_d.ap())
            nc.sync.dma_start(ident[:], i_d.ap())

            def body():
                ot_flip = [0]
                for g0, gn in groups:
                    # ost free layout: (co, hc, j, op)
                    ost = ost_pool.tile([128, C * HC * G * S], dt.float32,
                                        tag="ost")
                    ost_v = ost[:].rearrange("p (co hc j op) -> p co hc j op",
                                             co=C, hc=HC, op=S)
                    for jl in range(gn):
                        jw = g0 + jl
                        wlo = jw * S - 1
                        q_lo = max(0, -wlo)
                        q_hi = min(QN, W - wlo)

                        # ---- load [h 128, (hc, q, ci)] ----
                        itile = inp_pool.tile([128, HC * QN * C], dt.float32,
                                              tag="itile")
                        it_v = itile[:].rearrange(
                            "p (hc q ci) -> p hc q ci", hc=HC, ci=C)
                        if q_hi - q_lo < QN:
                            nc.vector.memset(itile[:], 0.0)
                        nc.sync.dma_start(
                            it_v[:, :, q_lo:q_hi, :],
                            x_v[:, :, wlo + q_lo:wlo + q_hi, :])

                        # ---- transposes -> K-tile [128=(q,ci), 1+H+1] ----
                        kt = kt_pool.tile([128, H + 2], dt.float32r, tag="kt")
                        nc.vector.memset(kt[:, 0:1].bitcast(dt.float32), 0.0)
                        nc.vector.memset(kt[:, H + 1:H + 2].bitcast(dt.float32), 0.0)
                        for mc in range(MMC):
                            tp = tr_ps.tile([128, 512], dt.float32, tag="tp")
                            for c in range(4):
                                hc = mc * 4 + c
                                nc.tensor.transpose(
                                    tp[:, c * 128:(c + 1) * 128],
                                    itile[:, hc * 128:(hc + 1) * 128],
                                    ident[:])
                            nc.vector.tensor_copy(
                                kt[:, 1 + mc * 512:1 + (mc + 1) * 512], tp[:])

                        # ---- conv + staging + out-transpose ----
                        for mc in range(MMC):
                            pm = mm_ps.tile([M, 512], dt.float32, tag="pm")
                            for ky in range(KS):
                                nc.tensor.matmul(
                                    pm[:],
                                    tmat[:, ky * M:(ky + 1) * M],
                                    kt[:, mc * 512 + ky:mc * 512 + ky + 512],
                                    start=(ky == 0), stop=(ky == KS - 1))
                            st = st_pool.tile([M, 512], dt.float32, tag="st")
                            nc.scalar.copy(st[:], pm[:])
                            po = ot_ps.tile([128, 4 * M], dt.float32, tag="po")
                            for c in range(4):
                                nc.tensor.transpose(
                                    po[:, c * M:(c + 1) * M],
                                    st[:, c * 128:(c + 1) * 128],
                                    ident[0:M, 0:M])
                            # po [p, (c, co, op)] -> ost [p, co, hc=mc*4+c, jl, op]
                            po_v = po[:].rearrange(
                                "p (c co op) -> p co c op", co=C, op=S)
                            dst = ost_v[:, :, mc * 4:(mc + 1) * 4, jl, :]
                            if ot_flip[0] % 2 == 0:
                                nc.vector.tensor_copy(dst, po_v)
                            else:
                                nc.scalar.copy(dst, po_v)
                            ot_flip[0] += 1

                    # ---- flush group to NCHW output ----
                    w0 = g0 * S
                    wn = min(W - w0, gn * S)
                    ost_w = ost[:].rearrange("p (co hc w) -> p hc co w",
                                             co=C, hc=HC)
                    for hc in range(HC):
                        nc.sync.dma_start(
                            y_v[:, hc, :, w0:w0 + wn],
                            ost_w[:, hc, :, 0:wn])

            if loop_count == 1:
                body()
            else:
                with tc.For_i(0, loop_count, 1):
                    body()

    nc.compile()
    return nc


def _toeplitz_weights(Wk):
    """Wk [3,3,ci,co] HWIO -> T [128, 3*96]; T[q*16+ci, ky*96+co*6+op] = Wk[ky, q-op, ci, co]."""
    T = np.zeros((128, KS * M), np.float32)
    for ky in range(KS):
        for op in range(S):
            for kx in range(KS):
                q = op + kx
                rows = slice(q * C, (q + 1) * C)
                T[rows, ky * M + op:ky * M + M:S] = Wk[ky, kx]
    return T


_CACHED = {}


def _get_program(H, W, n_cores):
    key = (H, W, n_cores)
    if key not in _CACHED:
        _CACHED[key] = _build_conv_program(H, W, n_cores)
    return _CACHED[key]


def kernel(x: np.ndarray, W: np.ndarray) -> np.ndarray:
    from concourse.bass_utils import run_bass_kernel_spmd

    B, Cc, H, Wd = x.shape
    assert Cc == C
    x_nhwc = np.ascontiguousarray(x).reshape(B, H, Wd * C)
    T = _toeplitz_weights(np.asarray(W, np.float32))
    ident = np.eye(128, dtype=np.float32)

    nc = _get_program(H, Wd, B)
    in_maps = [{"x": x_nhwc[b], "tmat": T, "ident": ident} for b in range(B)]
    res = run_bass_kernel_spmd(nc, in_maps, list(range(B)))
    y = np.stack([res.results[b]["y"] for b in range(B)], axis=0)
    return y.astype(np.float32, copy=False)



# revision 3
# speedup vs baseline: 1.1390x; 1.1390x over previous
"""Trainium2 Bass kernel for 3x3 SAME conv: B=8, Cin=Cout=16, 1024x1024, fp32.

Reference semantics:
  x (8,16,1024,1024) fp32 raw-reshaped to NHWC (8,1024,1024,16);
  y = conv2d_3x3_SAME(x_nhwc, W[3,3,16,16] HWIO) -> NCHW (8,16,1024,1024).

Per-core (batch-parallel, image b -> core b), bf16 compute (tolerance 2e-2 >>
bf16 error ~4e-3; fp16 PE-transpose faults on this HW, bf16 verified exact):
  - Input loaded per group of GJ=19 out-blocks as one [h 128, (hc, w 116, ci)]
    bf16 tile (3.7KB runs, 1-col halo -> 1.02x read amplification), prefetched
    one hc-chunk per iteration ~17 blocks ahead.
  - K-tiles [(q,ci)=128 part, 1+h+1 free] via 8 PE transposes (bf16 1 cyc/row)
    into two half-bank PSUM tiles, 2 DVE copies into a 6-deep persistent ring
    (edge zeros written once).
  - Conv = ky-outer accumulation: 3 bf16 matmuls per 512-col chunk, lhsT =
    Toeplitz T_ky [128, 96=(co*6+op)], rhs = K-tile shifted ky-1 in h. One
    2-bank fp32 PSUM tile per block.
  - Output: one wide ACT copy psum->bf16 SBUF [96,1024], single DMA per block
    to DRAM y[jw, 96, h] (2KB runs, single partition dim -- a split partition
    dim in the DMA AP silently corrupts). Host reorders to NCHW fp32.
  Measured ~303 us/core on HW (L=101 loop bench), cost-model 309 us; baseline
  was 885 us (sim). PE-bound (94% busy).
"""
import numpy as np

C = 16          # channels
S = 6           # out-block width
QN = 8          # in-block width
KS = 3
M = 96          # = C * S
GJ = 19         # out-blocks per input group
NG = 9          # groups (171 = 9*19)

LAST_RESULT = None


def _build_conv_program(H, W, n_cores, loop_count=1):
    import concourse.bacc as bacc
    import concourse.tile as tile
    import concourse.mybir as mybir

    dt = mybir.dt
    JW = (W + S - 1) // S
    assert JW == NG * GJ
    HC = H // 128
    assert H % 512 == 0
    MMC = H // 512
    WG = GJ * S + 2                 # input cols per group (incl 1-col halo each side)
    WO = JW * S                     # padded output width (1026)

    nc = bacc.Bacc("TRN2", target_bir_lowering=False, debug=False,
                   num_devices=n_cores)
    x_d = nc.dram_tensor("x", [H, W * C], dt.bfloat16, kind="ExternalInput")
    t_d = nc.dram_tensor("tmat", [128, KS * M], dt.bfloat16, kind="ExternalInput")
    i_d = nc.dram_tensor("ident", [128, 128], dt.bfloat16, kind="ExternalInput")
    y_d = nc.dram_tensor("y", [JW, M, H], dt.bfloat16, kind="ExternalOutput")
    # x as [p, hc, w, ci]
    x_v = x_d.ap().rearrange("(hc p) (w ci) -> p hc w ci", p=128, ci=C)
    # y as [j, m=(co*6+op), h] -- single partition dim per out-DMA
    y_v = y_d.ap()

    NKT = 6                         # persistent K-tile ring depth

    with tile.TileContext(nc) as tc:
        with tc.tile_pool(name="const", bufs=1) as cpool, \
             tc.tile_pool(name="inp", bufs=2) as inp_pool, \
             tc.tile_pool(name="stp", bufs=8) as st_pool, \
             tc.tile_pool(name="tr_ps", bufs=2, space="PSUM") as tr_ps, \
             tc.tile_pool(name="mm_ps", bufs=2, space="PSUM") as mm_ps:

            tmat = cpool.tile([128, KS * M], dt.bfloat16)
            ident = cpool.tile([128, 128], dt.bfloat16)
            nc.sync.dma_start(tmat[:], t_d.ap())
            nc.sync.dma_start(ident[:], i_d.ap())

            # Persistent K-tile ring; edge zeros (h=-1 / h=H) written once.
            kts = [cpool.tile([128, H + 2], dt.bfloat16, tag=f"kt{i}",
                              name=f"kt{i}")
                   for i in range(NKT)]
            for kt in kts:
                nc.vector.memset(kt[:, 0:1], 0.0)
                nc.vector.memset(kt[:, H + 1:H + 2], 0.0)

            gts = {}
            SKEW = 2                # software-pipeline distance (tr ahead of mm)

            def load_group(g, hc_list):
                base = g * GJ * S - 1       # x col of gtile w_local 0
                lo = max(0, -base)
                hi = min(WG, W - base)
                if g not in gts:
                    gt = inp_pool.tile([128, HC * WG * C], dt.bfloat16,
                                       tag="gt", name=f"gt{g}")
                    gts[g] = gt
                gt = gts[g]
                gt_v = gt[:].rearrange("p (hc w ci) -> p hc w ci",
                                       hc=HC, ci=C)
                for hc in hc_list:
                    if lo > 0:
                        nc.vector.memset(gt_v[:, hc:hc + 1, 0:lo, :], 0.0)
                    if hi < WG:
                        nc.vector.memset(gt_v[:, hc:hc + 1, hi:WG, :], 0.0)
                    nc.sync.dma_start(
                        gt_v[:, hc:hc + 1, lo:hi, :],
                        x_v[:, hc:hc + 1, base + lo:base + hi, :])

            def stage_tr(jw):
                g, jl = divmod(jw, GJ)
                if jw == 0:
                    load_group(0, range(HC))
                if 2 <= jl < 2 + HC and g + 1 < NG:
                    load_group(g + 1, [jl - 2])   # spread prefetch, 1 hc/iter
                gt = gts[g]
                # ---- K-tile [128=(q,ci), 1+H+1] via PE transposes ----
                kt = kts[jw % NKT]
                for half in range(2):
                    tp = tr_ps.tile([128, H // 2], dt.bfloat16,
                                    tag=f"tp{half}", name=f"tp{jw}_{half}")
                    for c in range(HC // 2):
                        hc = half * (HC // 2) + c
                        c0 = (hc * WG + jl * S) * C
                        nc.tensor.transpose(
                            tp[:, c * 128:(c + 1) * 128],
                            gt[:, c0:c0 + QN * C],
                            ident[:])
                    nc.vector.tensor_copy(
                        kt[:, 1 + half * 512:1 + (half + 1) * 512], tp[:])

            def stage_mm(jw):
                kt = kts[jw % NKT]
                # ---- conv (ky-outer over one 2-bank psum tile) ----
                pm = mm_ps.tile([M, H], dt.float32, tag="pm",
                                name=f"pm_{jw}")
                for ky in range(KS):
                    for mc in range(MMC):
                        nc.tensor.matmul(
                            pm[:, mc * 512:(mc + 1) * 512],
                            tmat[:, ky * M:(ky + 1) * M],
                            kt[:, mc * 512 + ky:mc * 512 + ky + 512],
                            start=(ky == 0), stop=(ky == KS - 1))
                # ---- staging: one wide ACT copy ----
                st = st_pool.tile([M, H], dt.bfloat16, tag="st",
                                  name=f"st{jw}")
                nc.scalar.copy(st[:], pm[:])
                nc.sync.dma_start(y_v[jw], st[:])

            def body():
                for jw in range(JW + SKEW):
                    if jw < JW:
                        stage_tr(jw)
                    if jw >= SKEW:
                        stage_mm(jw - SKEW)
                gts.clear()

            if loop_count == 1:
                body()
            else:
                with tc.For_i(0, loop_count, 1):
                    body()

    nc.compile()
    return nc


def _toeplitz_weights(Wk):
    """Wk [3,3,ci,co] HWIO -> T [128, 3*96]; T[q*16+ci, ky*96+co*6+op] = Wk[ky, q-op, ci, co]."""
    T = np.zeros((128, KS * M), np.float32)
    for ky in range(KS):
        for op in range(S):
            for kx in range(KS):
                q = op + kx
                rows = slice(q * C, (q + 1) * C)
                T[rows, ky * M + op:ky * M + M:S] = Wk[ky, kx]
    return T


_CACHED = {}


def _get_program(H, W, n_cores):
    key = (H, W, n_cores)
    if key not in _CACHED:
        _CACHED[key] = _build_conv_program(H, W, n_cores)
    return _CACHED[key]


def kernel(x: np.ndarray, W: np.ndarray) -> np.ndarray:
    from concourse.bass_utils import run_bass_kernel_spmd

    B, Cc, H, Wd = x.shape
    assert Cc == C
    import ml_dtypes
    bf16 = ml_dtypes.bfloat16
    x16 = np.ascontiguousarray(x).reshape(B, H, Wd * C).astype(bf16)
    T = _toeplitz_weights(np.asarray(W, np.float32)).astype(bf16)
    ident = np.eye(128, dtype=bf16)

    nc = _get_program(H, Wd, B)
    in_maps = [{"x": x16[b], "tmat": T, "ident": ident} for b in range(B)]
    res = run_bass_kernel_spmd(nc, in_maps, list(range(B)))
    global LAST_RESULT
    LAST_RESULT = res
    JW = (Wd + S - 1) // S
    y = np.empty((B, C, H, Wd), np.float32)
    for b in range(B):
        y2 = np.asarray(res.results[b]["y"])     # [JW, 96, H] bf16
        for j in range(JW):
            blk = y2[j].reshape(C, S, H).transpose(0, 2, 1)   # [co, h, op]
            w0 = j * S
            wn = min(S, Wd - w0)
            y[b][:, :, w0:w0 + wn] = blk[:, :, :wn]
    return y
